# revision 1
# baseline (speedup 1.0000x reference)
"""TRN2 Bass kernel for nn_MultiHeadAttention_50835232916148.

Pre-LN MHA block (HS=1024, 16 heads, bs=8, sl=1024), data-parallel over
batch across 8 NeuronCores (bs=1 per core, no collectives).

Key design points (vs a straightforward fp32 implementation):
- All matmuls run in bf16 (weights host-cast; x also shipped pre-cast as
  `xbd`), enabling fast weight loads; the fp32 x is read only for the
  final residual add.  PSUM accumulation is fp32 throughout and the
  residual is added in fp32, so end-to-end rel err stays ~3e-4.
- Scores for the two heads of a pair are computed CONCURRENTLY via
  row-tiled K=64 matmuls (tile_position (0,0)/(64,0)) into the two
  banks of one wide [128,1024] PSUM tile.
- Softmax exp runs as ONE wide [128,1024] activation per score tile,
  with the key-padding mask folded into the exp's per-partition bias AP
  (mask handling costs zero extra instructions).
- The softmax denominator rides the ctx matmul as a 65th lhsT column;
  the reciprocal is computed batched (Ln/Exp) after a DRAM gather and
  broadcast back over 64 partitions with a stride-0 DRAM->SBUF DMA.
- Each head pair's q/k projections are software-pipelined INTO the
  previous pair's attention loop (the acc-pool PSUM banks freed by the
  ctx drain host them), so the PE never idles while the exp stream
  paces the inner loop; head pair 7 hides the k=0..6 partial chains of
  out-proj blocks j=0/1 the same way.
- ctx PSUM banks are drained to SBUF bf16 immediately so the (long,
  latency-tolerant) normalize chain never blocks bank reuse.
- Dynamic-DMA issue costs ~0.6us per descriptor on an engine queue, so
  loads are spread across the sync/gpsimd/scalar queues with the
  critical x tiles issued first.

Per-core dataflow ([feature, token] transposed activations):
  xb (bf16, from host) --ones-matmul stats--> istd, b2
  y = xb*bcast(istd) + bcast(b2)                      [d,t] bf16
  vaug[t, h*65:(h+1)*65] = [y.T@Wv + bv | 1]          token-major
  per head-pair hp (heads A=2hp, B=2hp+1):
    qb,kb = WqkT.T @ y + b      (interleaved into hp-1's loop)
    per (n, jt): scoresT = [kbA^T qbA | kbB^T qbB]    wide 2-bank PSUM
                 pt = exp(scoresT + mask[jt])         one wide ACT
                 ctx_aug[hh][n] += vaug_h[jt].T @ pt-half
    recip = exp(-ln(denom)) via DRAM gather; rb = bcast DMA
    ctxn = ctx_aug[0:64] * rb  (head B shifted to partitions 64-127)
  outT = WoutT.T @ ctxn; out = (outT + bo) + xT       fused DVE stt
"""

import numpy as np

import concourse.bass as bass
import concourse.mybir as mybir
import concourse.tile as tile
from concourse.bass_utils import run_bass_kernel_spmd

P = 128
HS = 1024
SL = 1024
NHEAD = 16
DH = 64
BS = 8
NT = HS // P          # 8 feature/token tiles
TC = 512              # matmul free-dim chunk (fp32 PSUM bank)
NCH = SL // TC        # 2
LN_EPS = 1e-5
MASK_NEG = -1e8
F32 = mybir.dt.float32
F32R = mybir.dt.float32r
BF16 = mybir.dt.bfloat16
AF = mybir.ActivationFunctionType
ALU = mybir.AluOpType


def _hoist_waits(nc):
    """walrus in this env rejects >1 inline wait per instruction and ANY
    inline wait on Matmult; hoist them onto single-wait NoOps."""
    n_fixed = 0
    for _, bb in nc.bb_map.items():
        inner = bb.bb
        insts = inner.instructions
        new = []
        changed = False
        for inst in insts:
            si = getattr(inst, "sync_info", None)
            if si is not None and si.on_wait:
                keep = 0 if isinstance(inst, mybir.InstMatmult) else 1
                waits = list(si.on_wait)
                if len(waits) > keep:
                    kept = waits[-keep:] if keep else []
                    for w in waits[: len(waits) - keep]:
                        new.append(
                            mybir.InstNoOp(
                                name=nc.get_next_instruction_name(),
                                sync_info=mybir.SyncInfo(on_wait=[w], on_update=[]),
                                bass_nofuse=True,
                                engine=inst.engine,
                            )
                        )
                    inst.sync_info = mybir.SyncInfo(
                        on_wait=kept, on_update=list(si.on_update)
                    )
                    n_fixed += 1
                    changed = True
            new.append(inst)
        if changed:
            inner.instructions = new
    return n_fixed


def _build_nc(hoist=True):
    nc = bass.Bass()

    xt = nc.dram_tensor("xt", [HS, SL], F32R, kind="ExternalInput")
    xbd = nc.dram_tensor("xbd", [HS, SL], BF16, kind="ExternalInput")
    wqkv = nc.dram_tensor("wqkv", [HS, 3 * HS], BF16, kind="ExternalInput")
    wout = nc.dram_tensor("wout", [HS, HS], BF16, kind="ExternalInput")
    bqk = nc.dram_tensor("bqk", [P, 16], F32, kind="ExternalInput")
    bvb = nc.dram_tensor("bvb", [P, HS], BF16, kind="ExternalInput")
    bo = nc.dram_tensor("bo", [P, NT], F32, kind="ExternalInput")
    mb = nc.dram_tensor("mb", [P, NT], F32, kind="ExternalInput")
    onesr = nc.dram_tensor("onesr", [1, P], BF16, kind="ExternalInput")
    onesc = nc.dram_tensor("onesc", [P, 1], BF16, kind="ExternalInput")
    epsr = nc.dram_tensor("epsr", [1, 1], F32, kind="ExternalInput")
    out = nc.dram_tensor("out", [HS, SL], F32, kind="ExternalOutput")
    scio = nc.dram_tensor("scio", [1, 2 * SL], BF16, kind="Internal")
    # per-(hp, half) DRAM scratch for the softmax denominator reciprocal
    sden = [[nc.dram_tensor(f"sden{h}_{n}", [1, 2 * TC], BF16, kind="Internal")
             for n in range(NCH)] for h in range(NHEAD // 2)]
    srec = [[nc.dram_tensor(f"srec{h}_{n}", [2, TC], BF16, kind="Internal")
             for n in range(NCH)] for h in range(NHEAD // 2)]

    with tile.TileContext(nc) as tc, nc.allow_low_precision(
            reason="bf16 matmuls; tolerance is 2e-2 and residual is fp32"):
        with (
            tc.tile_pool(name="big", bufs=1) as big,
            tc.tile_pool(name="wstream", bufs=6) as wstream,
            tc.tile_pool(name="wvs", bufs=16) as wvs,
            tc.tile_pool(name="scratch", bufs=2) as scratch,
            tc.tile_pool(name="qks", bufs=3) as qks,
            tc.tile_pool(name="pts", bufs=3) as pts,
            tc.tile_pool(name="stream", bufs=3) as stream,
            tc.tile_pool(name="vecs", bufs=1) as vecs,
            tc.tile_pool(name="rbp", bufs=4) as rbp,
            tc.tile_pool(name="denp", bufs=2) as denp,
            tc.tile_pool(name="consts", bufs=1) as consts,
            tc.tile_pool(name="wide", bufs=2, space="PSUM") as wide,
            tc.tile_pool(name="acc", bufs=4, space="PSUM") as acc,
        ):
            # ---- big activation tiles ----
            t_y = big.tile([P, NT, SL], BF16, tag="y")
            xbs = [big.tile([P, SL], BF16, tag=f"xb{i}", name=f"xb{i}")
                   for i in range(NT)]

            # ACT table warmup first: memset-fed exp has no DMA deps, so
            # the ~2.7us table load runs during the framework preamble
            t_warm = vecs.tile([1, 1], F32, tag="warm")
            nc.vector.memset(t_warm[:], 1.0)
            nc.scalar.activation(t_warm[:], t_warm[:], AF.Exp)

            # ================= Phase 1: LayerNorm =================
            # xbd loads go first: the sync engine issues dynamic DMAs at
            # only ~1.5/us, so consts ride the scalar queue instead.
            st_m = wide.tile([1, SL], F32, tag="wide", name="stm")
            st_s = wide.tile([1, SL], F32, tag="wide", name="sts")
            dma_engs = [nc.sync, nc.gpsimd]
            c_oc = consts.tile([P, 1], BF16, tag="onesc")
            nc.scalar.dma_start(c_oc[:], onesc[:])
            for i in range(NT):
                dma_engs[i % 2].dma_start(xbs[i][:],
                                          xbd[i * P:(i + 1) * P, :])
            # warm the PE clock (HAM) during the DMA window so the LN
            # stats matmuls don't run at the cold half-rate
            ps_warm = acc.tile([1, TC], F32, tag="acc", name="warmps")
            for w in range(60):
                nc.tensor.matmul(ps_warm[:, 0:1], c_oc[:], c_oc[:],
                                 start=True, stop=True)
            c_bqk = consts.tile([P, 16], F32, tag="bqk")
            nc.scalar.dma_start(c_bqk[:], bqk[:])
            c_bvb = consts.tile([P, HS], BF16, tag="bvb")
            c_bo = consts.tile([P, NT], F32, tag="bo")
            nc.scalar.dma_start(c_bo[:], bo[:])
            c_mb = consts.tile([P, NT], F32, tag="mb")
            nc.scalar.dma_start(c_mb[:], mb[:])
            c_or = consts.tile([1, P], BF16, tag="onesr")
            nc.scalar.dma_start(c_or[:], onesr[:])
            c_or65 = consts.tile([DH + 1, P], BF16, tag="or65")
            nc.scalar.dma_start(c_or65[DH:DH + 1, :], onesr[:])
            c_eps = consts.tile([1, 1], F32, tag="eps")
            nc.scalar.dma_start(c_eps[:], epsr[:])
            for i in range(NT):
                sq = scratch.tile([P, SL], BF16, tag="sq")
                nc.vector.tensor_mul(sq[:], xbs[i][:], xbs[i][:])
                for n in range(NCH):
                    sl_ = slice(n * TC, (n + 1) * TC)
                    nc.tensor.matmul(st_m[:, sl_], c_oc[:], xbs[i][:, sl_],
                                     start=(i == 0), stop=(i == NT - 1))
                    nc.tensor.matmul(st_s[:, sl_], c_oc[:], sq[:, sl_],
                                     start=(i == 0), stop=(i == NT - 1))
            # LN tail chunked per 512-column half: halves every serial
            # step on the y critical path; the bcast matmuls live in the
            # (free) acc pool so V-proj's wide-PSUM slots unblock at the
            # stats copies, not after the whole chain
            v_mean = vecs.tile([1, SL], F32, tag="mean")
            v_msq = vecs.tile([1, SL], F32, tag="msq")
            v_tmp = vecs.tile([1, SL], F32, tag="tmp")
            v_lnv = vecs.tile([1, SL], F32, tag="lnv")
            v_istd = vecs.tile([1, SL], BF16, tag="istd")
            v_b2 = vecs.tile([1, SL], BF16, tag="b2")
            t_A = scratch.tile([P, SL], BF16, tag="ab", name="tA")
            t_B = scratch.tile([P, SL], BF16, tag="ab", name="tB")
            for c in range(NCH):
                cl = slice(c * TC, (c + 1) * TC)
                nc.scalar.activation(v_msq[:, cl], st_s[:, cl], AF.Copy,
                                     scale=1.0 / HS)
                nc.scalar.activation(v_mean[:, cl], st_m[:, cl], AF.Copy,
                                     scale=1.0 / HS)
                nc.vector.tensor_mul(v_tmp[:, cl], v_mean[:, cl],
                                     v_mean[:, cl])
                nc.vector.tensor_sub(v_msq[:, cl], v_msq[:, cl],
                                     v_tmp[:, cl])   # -> var
                nc.scalar.activation(v_lnv[:, cl], v_msq[:, cl], AF.Ln,
                                     bias=c_eps[:])
                nc.scalar.activation(v_istd[:, cl], v_lnv[:, cl], AF.Exp,
                                     scale=-0.5)
                nc.vector.scalar_tensor_tensor(v_b2[:, cl], v_mean[:, cl],
                                               -1.0, v_istd[:, cl],
                                               ALU.mult, ALU.mult)
                pA = acc.tile([P, TC], F32, tag="acc", name=f"pA{c}")
                nc.tensor.matmul(pA[:], c_or[:], v_istd[:, cl],
                                 start=True, stop=True)
                nc.vector.tensor_copy(t_A[:, cl], pA[:])
                pB = acc.tile([P, TC], F32, tag="acc", name=f"pB{c}")
                nc.tensor.matmul(pB[:], c_or[:], v_b2[:, cl],
                                 start=True, stop=True)
                nc.vector.tensor_copy(t_B[:, cl], pB[:])
                for i in range(NT):
                    t1 = stream.tile([P, TC], BF16, tag="t1", bufs=4,
                                     name=f"yt{c}_{i}")
                    nc.vector.tensor_mul(t1[:], xbs[i][:, cl], t_A[:, cl])
                    nc.vector.tensor_add(t_y[:, i, cl], t1[:], t_B[:, cl])

            # ============ Phase 2: V projection (token layout) ==========
            nc.sync.dma_start(c_bvb[:], bvb[:])
            t_vaug = big.tile([P, NT, NHEAD * (DH + 1)], BF16, tag="vaug")
            wv_tiles = []
            for k in range(NT):
                for n in range(NCH):
                    wv = wvs.tile([P, TC], BF16, tag="wv", name=f"wv{k}_{n}")
                    [nc.sync, nc.gpsimd, nc.scalar][(k * NCH + n) % 3].dma_start(
                        wv[:], wqkv[k * P:(k + 1) * P,
                                    2 * HS + n * TC:2 * HS + (n + 1) * TC])
                    wv_tiles.append(wv)
            for i in range(NT):
                ps_w = wide.tile([P, SL], F32, tag="wide", name=f"vps{i}")
                for n in range(NCH):
                    for k in range(NT):
                        nc.tensor.matmul(
                            ps_w[:, n * TC:(n + 1) * TC],
                            t_y[:, k, i * P:(i + 1) * P],
                            wv_tiles[k * NCH + n][:],
                            start=(k == 0), stop=(k == NT - 1))
                dst = t_vaug[:, i, :].rearrange("p (h c) -> p h c", c=DH + 1)
                nc.vector.tensor_add(
                    dst[:, :, 0:DH],
                    ps_w[:].rearrange("p (h c) -> p h c", c=DH),
                    c_bvb[:].rearrange("p (h c) -> p h c", c=DH))
                nc.vector.memset(dst[:, :, DH:DH + 1], 1.0)

            # ========= Phase 3+4: per head-pair QK proj + attention =====
            t_ctxn = big.tile([P, NT, SL], BF16, tag="ctxn")

            def normalize(hp, allp, eng=None):
                eng = eng or nc.sync
                # batched recip: gather all 4 denominator rows through DRAM
                # onto 4 partitions so one Ln + one Exp covers them all
                for dn in range(NCH):
                    sd = sden[hp][dn]
                    for hh in range(2):
                        eng.dma_start(sd[0:1, hh * TC:(hh + 1) * TC],
                                          allp[dn][hh][DH:DH + 1, :])
                t_d4 = denp.tile([4, TC], BF16, tag="d4", name=f"d4{hp}")
                for dn in range(NCH):
                    eng.dma_start(
                        t_d4[2 * dn:2 * dn + 2, :],
                        sden[hp][dn][:].rearrange("o (r c) -> (o r) c", c=TC))
                t_l4 = denp.tile([4, TC], F32, tag="l4", name=f"l4{hp}")
                nc.scalar.activation(t_l4[:], t_d4[:], AF.Ln)
                t_r4 = denp.tile([4, TC], BF16, tag="r4", name=f"r4{hp}")
                nc.scalar.activation(t_r4[:], t_l4[:], AF.Exp, scale=-1.0)
                for dn in range(NCH):
                    eng.dma_start(srec[hp][dn][:],
                                      t_r4[2 * dn:2 * dn + 2, :])
                for dn in range(NCH):
                    sl_ = slice(dn * TC, (dn + 1) * TC)
                    for hh in range(2):
                        rb = rbp.tile([DH, TC], BF16, tag="rb",
                                      name=f"rb{hp}_{hh}_{dn}")
                        eng.dma_start(
                            rb[:],
                            srec[hp][dn][hh:hh + 1, :].broadcast_to((DH, TC)))
                        if hh == 0:
                            nc.vector.tensor_mul(t_ctxn[0:DH, hp, sl_],
                                                 allp[dn][hh][0:DH, :], rb[:])
                        else:
                            cs = rbp.tile([DH, TC], BF16, tag="cs",
                                          name=f"cs{hp}_{dn}")
                            nc.vector.tensor_mul(cs[:], allp[dn][hh][0:DH, :],
                                                 rb[:])
                            eng.dma_start(t_ctxn[DH:P, hp, sl_], cs[:])

            def build_proj_steps(hp):
                """Prefetch wj DMAs for head-pair hp and return
                (qb, kb, [n0_steps, n1_steps]) where each steps list holds
                closures emitting one proj matmul (or the bias op) each,
                to be interleaved into the previous hp's attention loop."""
                qb = qks.tile([P, SL], BF16, tag="qk", name=f"qb{hp}")
                kb = qks.tile([P, SL], BF16, tag="qk", name=f"kb{hp}")
                halves = []
                for blk, dstt in ((hp, qb), (8 + hp, kb)):
                    wj = wstream.tile([P, NT, P], BF16, tag="wqk",
                                      name=f"wj{blk}")
                    nc.sync.dma_start(
                        wj[:], wqkv[:, blk * P:(blk + 1) * P]
                        .rearrange("(n p) m -> p n m", p=P))
                    steps = []
                    for c in range(NCH):
                        box = {}

                        def mk_mm(i, c=c, blk=blk, wj=wj, box=box):
                            def f():
                                if i == 0:
                                    box["ps"] = acc.tile(
                                        [P, TC], F32, tag="acc",
                                        name=f"qk{blk}_{c}")
                                nc.tensor.matmul(
                                    box["ps"][:], wj[:, i, :],
                                    t_y[:, i, c * TC:(c + 1) * TC],
                                    start=(i == 0), stop=(i == NT - 1))
                            return f

                        for i in range(NT):
                            steps.append(mk_mm(i))

                        def mk_bias(c=c, blk=blk, dstt=dstt, box=box):
                            def f():
                                nc.vector.tensor_scalar_add(
                                    dstt[:, c * TC:(c + 1) * TC],
                                    box["ps"][:], c_bqk[:, blk:blk + 1])
                            return f

                        steps.append(mk_bias())
                    halves.append(steps)
                return qb, kb, halves

            # hp7's attention loop has no next-hp projections to hide, so
            # it hides the k=0..6 partial accumulations of out-proj blocks
            # j=0 (n0-half) and j=1 (n1-half); partials drain to SBUF and a
            # k=7 pass finishes them after the last normalize.
            out_wos = {}
            out_part = {}

            def build_out_steps(j):
                wo = wstream.tile([P, NT, P], BF16, tag="wqk", name=f"wo{j}")
                nc.sync.dma_start(
                    wo[:], wout[:, j * P:(j + 1) * P]
                    .rearrange("(n p) m -> p n m", p=P))
                out_wos[j] = wo
                steps = []
                for c in range(NCH):
                    box = {}

                    def mk_mm(k, c=c, j=j, wo=wo, box=box):
                        def f():
                            if k == 0:
                                box["ps"] = acc.tile(
                                    [P, TC], F32, tag="acc",
                                    name=f"opp{j}_{c}")
                            nc.tensor.matmul(
                                box["ps"][:], wo[:, k, :],
                                t_ctxn[:, k, c * TC:(c + 1) * TC],
                                start=(k == 0), stop=(k == NT - 2))
                        return f

                    for k in range(NT - 1):
                        steps.append(mk_mm(k))

                    def mk_drain(c=c, j=j, box=box):
                        def f():
                            pp = stream.tile([P, TC], F32, tag="opart",
                                             bufs=4, name=f"opart{j}_{c}")
                            nc.vector.tensor_copy(pp[:], box["ps"][:])
                            out_part[(j, c)] = pp
                        return f

                    steps.append(mk_drain())
                return steps

            qb_cur, kb_cur, halves0 = build_proj_steps(0)
            for st in halves0[0] + halves0[1]:
                st()
            pending = None
            for hp in range(NHEAD // 2):
                if hp < NHEAD // 2 - 1:
                    qb_nxt, kb_nxt, halves_nxt = build_proj_steps(hp + 1)
                else:
                    qb_nxt = kb_nxt = None
                    halves_nxt = [build_out_steps(0), build_out_steps(1)]

                if pending is not None:
                    normalize(*pending)
                    pending = None
                qb, kb = qb_cur, kb_cur
                drained = []
                ctx_ps = [[None] * NCH for _ in range(2)]
                ctxs = [[None] * NCH for _ in range(2)]

                # ctx matmuls trail the score/exp stream by one (jt, n)
                # step so their pt dependency is resolved long before issue
                def emit_ctx(djt, dn, dpt, hp=hp, ctx_ps=ctx_ps, ctxs=ctxs,
                             drained=drained):
                    for hh in range(2):
                        if djt == 0:
                            ctx_ps[hh][dn] = acc.tile(
                                [DH + 1, TC], F32, tag="acc",
                                name=f"ctx{hp}_{hh}_{dn}")
                        h = 2 * hp + hh
                        va = t_vaug[:, djt, h * (DH + 1):(h + 1) * (DH + 1)]
                        nc.tensor.matmul(
                            ctx_ps[hh][dn][:], va,
                            dpt[:, hh * TC:(hh + 1) * TC],
                            start=(djt == 0), stop=(djt == NT - 1))
                    if djt == NT - 1:
                        # free the two ctx banks of this chunk right away
                        for hh in range(2):
                            cs = rbp.tile([DH + 1, TC], BF16, tag="ctxs",
                                          bufs=8, name=f"ctxs{hp}_{hh}_{dn}")
                            nc.vector.tensor_copy(cs[:], ctx_ps[hh][dn][:])
                            ctxs[hh][dn] = cs
                        drained.append((hp, dn, [ctxs[0][dn], ctxs[1][dn]]))

                delayed = None
                for n in range(NCH):
                    inj = list(halves_nxt[n])
                    for jt in range(NT):
                        sl_ = slice(n * TC, (n + 1) * TC)
                        ps_s = wide.tile([P, SL], F32, tag="wide",
                                         name=f"s{hp}_{jt}_{n}")
                        nc.tensor.matmul(
                            ps_s[:, 0:TC],
                            kb[0:DH, jt * P:(jt + 1) * P],
                            qb[0:DH, sl_],
                            start=True, stop=True, tile_position=(0, 0))
                        nc.tensor.matmul(
                            ps_s[:, TC:2 * TC],
                            kb[DH:P, jt * P:(jt + 1) * P],
                            qb[DH:P, sl_],
                            start=True, stop=True, tile_position=(DH, 0))
                        pt = pts.tile([P, SL], BF16, tag="pt",
                                      name=f"pt{hp}_{jt}_{n}")
                        nc.scalar.activation(pt[:], ps_s[:], AF.Exp,
                                             bias=c_mb[:, jt:jt + 1])
                        for _ in range(3):
                            if inj:
                                inj.pop(0)()
                        if delayed is not None:
                            emit_ctx(*delayed)
                        delayed = (jt, n, pt)
                    for st in inj:
                        st()
                emit_ctx(*delayed)
                pending = (hp, [[ctxs[0][dn], ctxs[1][dn]] for dn in range(NCH)])
                qb_cur, kb_cur = qb_nxt, kb_nxt
            # last hp: low-latency normalize (ScalarE and the wide PSUM
            # pool are free at this point) — in-place Ln/Exp recip on the
            # drained ctx rows, K=1 matmul broadcast, no DRAM hops
            fhp, fallp = pending
            for dn in range(NCH):
                sl_ = slice(dn * TC, (dn + 1) * TC)
                ps_rb = wide.tile([P, SL], F32, tag="wide", name=f"rbps{dn}")
                for hh in range(2):
                    row = fallp[dn][hh][DH:DH + 1, :]
                    nc.scalar.activation(row, row, AF.Ln)
                    nc.scalar.activation(row, row, AF.Exp, scale=-1.0)
                    nc.tensor.matmul(ps_rb[0:DH, hh * TC:(hh + 1) * TC],
                                     c_or65[DH:DH + 1, 0:DH], row,
                                     start=True, stop=True,
                                     tile_position=(DH, 0))
                for hh in range(2):
                    rb = rbp.tile([DH, TC], BF16, tag="rb",
                                  name=f"rbf{fhp}_{hh}_{dn}")
                    nc.vector.tensor_copy(
                        rb[:], ps_rb[0:DH, hh * TC:(hh + 1) * TC])
                    if hh == 0:
                        nc.vector.tensor_mul(t_ctxn[0:DH, fhp, sl_],
                                             fallp[dn][hh][0:DH, :], rb[:])
                    else:
                        cs = rbp.tile([DH, TC], BF16, tag="cs",
                                      name=f"csf{fhp}_{dn}")
                        nc.vector.tensor_mul(cs[:], fallp[dn][hh][0:DH, :],
                                             rb[:])
                        nc.gpsimd.dma_start(t_ctxn[DH:P, fhp, sl_], cs[:])

            # ================= Phase 5: out-proj + residual =============
            wos, xrs = {0: out_wos[0], 1: out_wos[1]}, []
            for j in range(2, NT):
                wo = wstream.tile([P, NT, P], BF16, tag="wqk", name=f"wo{j}")
                nc.sync.dma_start(
                    wo[:], wout[:, j * P:(j + 1) * P]
                    .rearrange("(n p) m -> p n m", p=P))
                wos[j] = wo
            for j in range(NT):
                xr = stream.tile([P, SL], F32R, tag="xr", bufs=6,
                                 name=f"xr{j}")
                nc.scalar.dma_start(xr[:], xt[j * P:(j + 1) * P, :])
                xrs.append(xr)
            for j in [2, 3, 4, 5, 0, 1, 6, 7]:
                wo = wos[j]
                for n in range(NCH):
                    sl_ = slice(n * TC, (n + 1) * TC)
                    ot = stream.tile([P, TC], F32, tag="ot", bufs=4,
                                     name=f"ot{j}_{n}")
                    if j < 2:
                        # finish the hp7-hidden partial: k=7 term + partial
                        ps_o = acc.tile([P, TC], F32, tag="acc",
                                        name=f"opf{j}_{n}")
                        nc.tensor.matmul(ps_o[:], wo[:, NT - 1, :],
                                         t_ctxn[:, NT - 1, sl_],
                                         start=True, stop=True)
                        nc.vector.scalar_tensor_tensor(
                            ot[:], ps_o[:], c_bo[:, j:j + 1],
                            out_part[(j, n)][:], ALU.add, ALU.add)
                        nc.vector.tensor_add(ot[:], ot[:], xrs[j][:, sl_])
                    else:
                        ps_o = acc.tile([P, TC], F32, tag="acc",
                                        name=f"ops{j}_{n}")
                        for k in range(NT):
                            nc.tensor.matmul(ps_o[:], wo[:, k, :],
                                             t_ctxn[:, k, sl_],
                                             start=(k == 0), stop=(k == NT - 1))
                        nc.vector.scalar_tensor_tensor(ot[:], ps_o[:],
                                                       c_bo[:, j:j + 1],
                                                       xrs[j][:, sl_],
                                                       ALU.add, ALU.add)
                    [nc.sync, nc.gpsimd, nc.scalar][(2 * j + n) % 3].dma_start(
                        out[j * P:(j + 1) * P, sl_], ot[:])

    if hoist:
        _hoist_waits(nc)
    return nc


_NC_CACHE = None


def _get_nc():
    global _NC_CACHE
    if _NC_CACHE is None:
        _NC_CACHE = _build_nc()
    return _NC_CACHE


def _prep_in_maps(hidden_states, encoder_padding_mask, in_proj_weight,
                  in_proj_bias, out_proj_weight, out_proj_bias,
                  norm_weight, norm_bias):
    import ml_dtypes
    f = np.float32
    bf = ml_dtypes.bfloat16
    w2 = np.asarray(in_proj_weight, dtype=f).reshape(3 * HS, HS).copy()
    b2 = np.asarray(in_proj_bias, dtype=f).reshape(3 * HS).copy()
    # fold the LN affine (w, b) into the fused projection: W*(y*w+b)+bias
    # == (W*diag(w))*y + (bias + W@b)
    nw = np.asarray(norm_weight, dtype=f).reshape(HS)
    nb = np.asarray(norm_bias, dtype=f).reshape(HS)
    b2 = b2 + w2 @ nb
    w2 = w2 * nw[None, :]
    scale = f(1.0 / np.sqrt(DH))
    w2[0:HS] *= scale
    b2[0:HS] *= scale
    wqkv = np.ascontiguousarray(w2.T).astype(bf)           # [d, 3HS]
    wout = np.ascontiguousarray(
        np.asarray(out_proj_weight, dtype=f).T).astype(bf)
    bqk = np.ascontiguousarray(b2[:2 * HS].reshape(16, P).T)
    bvb = np.ascontiguousarray(
        np.broadcast_to(b2[2 * HS:], (P, HS))).astype(bf)
    bo = np.ascontiguousarray(np.asarray(out_proj_bias, dtype=f).reshape(NT, P).T)
    onesr = np.ones((1, P), dtype=bf)
    onesc = np.ones((P, 1), dtype=bf)
    epsr = np.full((1, 1), LN_EPS, f)
    shared = dict(wqkv=wqkv, wout=wout, bqk=bqk, bvb=bvb, bo=bo,
                  onesr=onesr, onesc=onesc, epsr=epsr)

    hs = np.asarray(hidden_states, dtype=f)
    mask = np.asarray(encoder_padding_mask)
    in_maps = []
    for c in range(BS):
        mbc = (mask[c].astype(f) * f(MASK_NEG)).reshape(NT, P).T
        xtc = np.ascontiguousarray(hs[c].T)
        in_maps.append(dict(
            xt=xtc,
            xbd=xtc.astype(bf),
            mb=np.ascontiguousarray(mbc),
            **shared,
        ))
    return in_maps


def _run(in_maps, trace=False):
    nc = _get_nc()
    return run_bass_kernel_spmd(nc, in_maps, list(range(BS)), trace=trace)


def kernel(**inputs):
    in_maps = _prep_in_maps(**inputs)
    res = _run(in_maps, trace=False)
    outs = [res.results[c]["out"].T for c in range(BS)]
    return np.stack(outs, axis=0).astype(np.float32)


def kernel_traced(**inputs):
    in_maps = _prep_in_maps(**inputs)
    res = _run(in_maps, trace=True)
    outs = [res.results[c]["out"].T for c in range(BS)]
    return np.stack(outs, axis=0).astype(np.float32), res.exec_time_ns

